# revision 7
# baseline (speedup 1.0000x reference)
"""Trainium2 Bass kernel for nn_DownSample (downsample conv + bidirectional
3-block Mamba stack).

Distribution: 8 cores = (batch 2) x (DI-quarter 4). Each core processes BOTH
directions (fwd + reversed) over the full downsampled length T=2048, owning 64
of the 256 inner channels. Cross-channel reductions (xproj over DI, outproj
over DI) are handled by 4-core AllGather collectives of the per-core channel
slices (xc and g). Everything else (residual stream, layernorm, outproj
matmul) is computed redundantly per core, which is cheap.

Self-contained: hardcodes all shapes; no sibling imports.
"""

import numpy as np

import concourse.bass as bass
import concourse.bacc as bacc
import concourse.mybir as mybir
import concourse.tile as tile
from concourse.bass_utils import run_bass_kernel_spmd

F32 = mybir.dt.float32
F16 = mybir.dt.float16
AF = mybir.ActivationFunctionType
OP = mybir.AluOpType

B = 2
CIN = 64
LX = 8192
D = 128          # d_model / conv out channels
T = 2048         # downsampled length
DI = 256         # d_inner
NST = 16         # d_state
DTR = 8          # dt_rank
NE = 64          # inner channels owned per core
NJ = NE // 8     # (d8, n16) tiles per core per (block, dir) = 8
NB = 3
N_CORES = 8
GROUPS = [[0, 1, 2, 3], [4, 5, 6, 7]]

_CACHED = {}


def _build_module():
    nc = bacc.Bacc("TRN2", target_bir_lowering=False, debug=False,
                   num_devices=N_CORES)

    # ---------------- dram I/O ----------------
    def din(name, shape, dt):
        return nc.dram_tensor(name, shape, dt, kind="ExternalInput").ap()

    x16 = din("x16", [CIN, LX], F16)
    wconv = din("wconv", [CIN, 4 * D], F16)
    convb = din("convb", [D, 1], F32)
    wxk = din("wxk", [D, 24 * NE], F16)      # (j,k) -> lhsT [D, NE]
    wz = din("wz", [D, 6 * NE], F16)
    wxp = din("wxp", [D, 12 * 40], F16)      # (j,et) -> [D, 40]
    wdt = din("wdt", [DTR, 6 * NE], F16)
    bdt = din("bdt", [NE, 6], F32)
    asc = din("asc", [D, 6 * NJ], F32)       # (j,jl) -> [128,1] A col
    dpp = din("dpp", [NE, 6], F32)
    lnw = din("lnw", [D, 6], F32)
    lnb = din("lnb", [D, 6], F32)
    wo = din("wo", [D, 12 * D], F16)         # (j,et) -> [D, D]
    cbb = din("cbb", [NE, 6], F32)
    selr = din("selr", [NE, NJ * 128], F16)
    selb = din("selb", [40, 128], F16)
    selc = din("selc", [40, 128], F16)
    sely = din("sely", [D, NJ * NE], F16)
    onesu = din("onesu", [1, D], F16)
    onesc = din("onesc", [D, 1], F16)
    oneb = din("oneb", [NE, 1], F32)
    epsb = din("epsb", [1, 1], F32)

    douts = {}
    for nm in ("yf", "rf", "yb", "rb"):
        douts[nm] = nc.dram_tensor(nm, [D, T], F32, kind="ExternalOutput").ap()

    with tile.TileContext(nc) as tc, \
         tc.tile_pool(name="wp", bufs=1) as wp, \
         tc.tile_pool(name="pers", bufs=1) as pers, \
         tc.tile_pool(name="xin", bufs=2) as xin, \
         tc.tile_pool(name="sb1", bufs=1) as sb1, \
         tc.tile_pool(name="sb2", bufs=2) as sb2, \
         tc.tile_pool(name="sbjj", bufs=3) as sbjj, \
         tc.tile_pool(name="sbsm", bufs=1) as sbsm, \
         tc.tile_pool(name="ps", bufs=4, space="PSUM") as ps, \
         tc.tile_pool(name="ps_y", bufs=1, space="PSUM") as ps_y, \
         tc.tile_pool(name="dram", bufs=2, space="DRAM") as dram:

        # ---------------- load persistent weights ----------------
        def wtile(ap_in, shape, dt):
            t = wp.tile(shape, dt, tag=ap_in.tensor.name + "_sb")
            nc.sync.dma_start(t[:], ap_in)
            return t

        wconv_sb = wtile(wconv, [CIN, 4 * D], F16)
        convb_sb = wtile(convb, [D, 1], F32)
        wxk_sb = wtile(wxk, [D, 24 * NE], F16)
        wz_sb = wtile(wz, [D, 6 * NE], F16)
        wxp_sb = wtile(wxp, [D, 12 * 40], F16)
        wdt_sb = wtile(wdt, [DTR, 6 * NE], F16)
        bdt_sb = wtile(bdt, [NE, 6], F32)
        asc_sb = wtile(asc, [D, 6 * NJ], F32)
        dp_sb = wtile(dpp, [NE, 6], F32)
        lnw_sb = wtile(lnw, [D, 6], F32)
        lnb_sb = wtile(lnb, [D, 6], F32)
        wo_sb = wtile(wo, [D, 12 * D], F16)
        cb_sb = wtile(cbb, [NE, 6], F32)
        selr_sb = wtile(selr, [NE, NJ * 128], F16)
        selb_sb = wtile(selb, [40, 128], F16)
        selc_sb = wtile(selc, [40, 128], F16)
        sely_sb = wtile(sely, [D, NJ * NE], F16)
        onesu_sb = wtile(onesu, [1, D], F16)
        onesc_sb = wtile(onesc, [D, 1], F16)
        oneb_sb = wtile(oneb, [NE, 1], F32)
        epsb_sb = wtile(epsb, [1, 1], F32)

        res = {0: pers.tile([D, T], F32, tag="res_f", name="res_f"),
               1: pers.tile([D, T], F32, tag="res_b", name="res_b")}

        # ---------------- prologue: downsample conv + silu ----------------
        for c in range(4):          # 512-wide output chunks
            xch = xin.tile([CIN, T], F16, tag="xch")
            nc.sync.dma_start(xch[:], x16[:, T * c:T * (c + 1)])
            cp = ps.tile([D, 512], F32, tag="b1", name="convps")
            for k in range(4):
                nc.tensor.matmul(cp[:], wconv_sb[:, k * D:(k + 1) * D],
                                 xch[:, k:k + 2045:4],
                                 start=(k == 0), stop=(k == 3))
            nc.scalar.activation(res[0][:, 512 * c:512 * (c + 1)], cp[:],
                                 AF.Silu, bias=convb_sb[:])
        # reversed copy for the backward direction
        nc.vector.tensor_copy(res[1][:], res[0][:, ::-1])

        # ---------------- blocks ----------------
        out_t = {0: None, 1: None}
        for i in range(NB):
            for d in (0, 1):
                j = 3 * d + i
                r = res[d]
                last = (i == NB - 1)

                # ---- layernorm stats (across partitions, via PE) ----
                m16 = sb1.tile([1, T], F16, tag="m16")
                rstd16 = sb1.tile([1, T], F16, tag="rstd16")
                for c in range(4):
                    cs = slice(512 * c, 512 * (c + 1))
                    r16 = sbsm.tile([D, 512], F16, tag="r16")
                    sq16 = sbsm.tile([D, 512], F16, tag="sq16")
                    nc.scalar.copy(r16[:], r[:, cs])
                    nc.scalar.activation(sq16[:], r[:, cs], AF.Square)
                    st0 = ps.tile([1, 512], F32, tag="b1", name="st0")
                    st1 = ps.tile([1, 512], F32, tag="b1", name="st1")
                    nc.tensor.matmul(st0[:], onesc_sb[:], r16[:],
                                     start=True, stop=True)
                    nc.tensor.matmul(st1[:], onesc_sb[:], sq16[:],
                                     start=True, stop=True)
                    msq = sbsm.tile([1, 512], F32, tag="msq")
                    nc.scalar.activation(msq[:], st0[:], AF.Square,
                                         scale=1.0 / 128.0)
                    nc.scalar.activation(m16[:, cs], st0[:], AF.Copy,
                                         scale=1.0 / 128.0)
                    var = sbsm.tile([1, 512], F32, tag="var")
                    nc.vector.scalar_tensor_tensor(
                        var[:], st1[:], 1.0 / 128.0, msq[:],
                        op0=OP.mult, op1=OP.subtract)
                    sd = sbsm.tile([1, 512], F32, tag="sd")
                    nc.scalar.activation(sd[:], var[:], AF.Sqrt,
                                         bias=epsb_sb[:])
                    rstd = sbsm.tile([1, 512], F32, tag="rstd")
                    nc.vector.reciprocal(rstd[:], sd[:])
                    nc.scalar.copy(rstd16[:, cs], rstd[:])

                # broadcast m/rstd along partitions and normalize
                hn = sb2.tile([D, 3 + T], F16, tag="hn")
                nc.vector.memset(hn[:, 0:3], 0.0)
                for c in range(4):
                    cs = slice(512 * c, 512 * (c + 1))
                    mb = ps.tile([D, 512], F32, tag="b1", name="mb")
                    rb_ = ps.tile([D, 512], F32, tag="b1", name="rb_")
                    nc.tensor.matmul(mb[:], onesu_sb[:], m16[:, cs],
                                     start=True, stop=True)
                    nc.tensor.matmul(rb_[:], onesu_sb[:], rstd16[:, cs],
                                     start=True, stop=True)
                    t1 = sbsm.tile([D, 512], F32, tag="t1")
                    nc.vector.tensor_tensor(t1[:], r[:, cs], mb[:],
                                            op=OP.subtract)
                    t2 = sbsm.tile([D, 512], F32, tag="t2")
                    nc.vector.tensor_tensor(t2[:], t1[:], rb_[:], op=OP.mult)
                    nc.scalar.activation(hn[:, 3 + 512 * c: 3 + 512 * (c + 1)],
                                         t2[:], AF.Identity,
                                         scale=lnw_sb[:, j:j + 1],
                                         bias=lnb_sb[:, j:j + 1])

                # ---- in_proj (+fused causal conv) for own channels ----
                xc16 = sb2.tile([NE, T], F16, tag="xc16")
                sz16 = sb2.tile([NE, T], F16, tag="sz16")
                for c in range(4):  # 512-wide chunks
                    t0 = 512 * c
                    cs = slice(t0, t0 + 512)
                    xm = ps.tile([NE, 512], F32, tag="b1", name="xm")
                    for k in range(4):
                        nc.tensor.matmul(
                            xm[:],
                            wxk_sb[:, (j * 4 + k) * NE:(j * 4 + k + 1) * NE],
                            hn[:, t0 + k: t0 + k + 512],
                            start=(k == 0), stop=(k == 3))
                    nc.scalar.activation(xc16[:, cs], xm[:], AF.Silu,
                                         bias=cb_sb[:, j:j + 1])
                    zp = ps.tile([NE, 512], F32, tag="b1", name="zp")
                    nc.tensor.matmul(zp[:], wz_sb[:, j * NE:(j + 1) * NE],
                                     hn[:, 3 + t0: 3 + t0 + 512],
                                     start=True, stop=True)
                    nc.scalar.activation(sz16[:, cs], zp[:], AF.Silu)

                # ---- AllGather xc ----
                ib = dram.tile([NE, T], F16, tag="ib_xc")
                ob = dram.tile([4 * NE, T], F16, tag="ob_xc")
                nc.gpsimd.dma_start(ib[:], xc16[:])
                nc.gpsimd.collective_compute(
                    "AllGather", OP.bypass, replica_groups=GROUPS,
                    ins=[ib.opt()], outs=[ob.opt()])
                xcf = [sb1.tile([D, T], F16, tag=f"xcf{et}",
                                name=f"xcf{et}") for et in range(2)]
                for et in range(2):
                    nc.gpsimd.dma_start(xcf[et][:],
                                        ob[128 * et:128 * (et + 1), :])

                # ---- xproj -> dbc ----
                dbc16 = sb1.tile([40, T], F16, tag="dbc16")
                for c in range(4):
                    cs = slice(512 * c, 512 * (c + 1))
                    dbp = ps.tile([40, 512], F32, tag="b1", name="dbp")
                    for et in range(2):
                        nc.tensor.matmul(dbp[:],
                                         wxp_sb[:, (j * 2 + et) * 40:
                                                (j * 2 + et + 1) * 40],
                                         xcf[et][:, cs],
                                         start=(et == 0), stop=(et == 1))
                    nc.scalar.copy(dbc16[:, cs], dbp[:])

                # ---- dt = softplus(dtproj @ dtlow + bdt) ----
                dt16 = sb2.tile([NE, T], F16, tag="dt16")
                for c in range(4):
                    cs = slice(512 * c, 512 * (c + 1))
                    dtp = ps.tile([NE, 512], F32, tag="b1", name="dtp")
                    nc.tensor.matmul(dtp[:], wdt_sb[:, j * NE:(j + 1) * NE],
                                     dbc16[0:DTR, cs],
                                     start=True, stop=True)
                    e16 = sbsm.tile([NE, 512], F16, tag="e16")
                    nc.scalar.activation(e16[:], dtp[:], AF.Exp,
                                         bias=bdt_sb[:, j:j + 1])
                    nc.scalar.activation(dt16[:, cs], e16[:], AF.Ln,
                                         bias=oneb_sb[:])

                # ---- u = dt * xc ----
                u16 = sb2.tile([NE, T], F16, tag="u16")
                nc.vector.tensor_tensor(u16[:], dt16[:], xc16[:], op=OP.mult)

                # ---- B_rep / C_rep ----
                b16 = sb2.tile([D, T], F16, tag="b16")
                c16 = sb2.tile([D, T], F16, tag="c16")
                for c in range(4):
                    cs = slice(512 * c, 512 * (c + 1))
                    bp = ps.tile([D, 512], F32, tag="b1", name="bp")
                    nc.tensor.matmul(bp[:], selb_sb[:], dbc16[:, cs],
                                     start=True, stop=True)
                    nc.scalar.copy(b16[:, cs], bp[:])
                    cp2 = ps.tile([D, 512], F32, tag="b1", name="cp2")
                    nc.tensor.matmul(cp2[:], selc_sb[:], dbc16[:, cs],
                                     start=True, stop=True)
                    nc.scalar.copy(c16[:, cs], cp2[:])

                # ---- per (d8,n16)-tile scan ----
                yp = ps_y.tile([NE, T], F32, tag="yps")
                for jl in range(NJ):
                    sel = selr_sb[:, jl * 128:(jl + 1) * 128]
                    da = sbjj.tile([D, T], F16, tag="da", bufs=2)
                    ur = sbjj.tile([D, T], F16, tag="ur", bufs=2)
                    for c in range(4):
                        cs = slice(512 * c, 512 * (c + 1))
                        dr = ps.tile([D, 512], F32, tag="b1", name="dr")
                        nc.tensor.matmul(dr[:], sel, dt16[:, cs],
                                         start=True, stop=True)
                        nc.scalar.activation(
                            da[:, cs], dr[:], AF.Exp,
                            scale=asc_sb[:, j * NJ + jl: j * NJ + jl + 1])
                        up = ps.tile([D, 512], F32, tag="b1", name="up")
                        nc.tensor.matmul(up[:], sel, u16[:, cs],
                                         start=True, stop=True)
                        nc.scalar.copy(ur[:, cs], up[:])
                    dbx = sbjj.tile([D, T], F16, tag="dbx")
                    nc.vector.tensor_tensor(dbx[:], ur[:], b16[:], op=OP.mult)
                    hh = sbjj.tile([D, T], F16, tag="hh", bufs=2)
                    nc.vector.tensor_tensor_scan(hh[:], da[:], dbx[:], 0.0,
                                                 op0=OP.mult, op1=OP.add)
                    # hC overwrites dbx
                    nc.vector.tensor_tensor(dbx[:], hh[:], c16[:], op=OP.mult)
                    for c in range(4):
                        cs = slice(512 * c, 512 * (c + 1))
                        nc.tensor.matmul(yp[:, cs],
                                         sely_sb[:, jl * NE:(jl + 1) * NE],
                                         dbx[:, cs],
                                         start=(jl == 0), stop=(jl == NJ - 1),
                                         skip_group_check=True)

                # ---- gate: g = (xc*Dp + y) * silu(z) ----
                g16 = sb1.tile([NE, T], F16, tag="g16")
                for c in range(2):
                    cs = slice(1024 * c, 1024 * (c + 1))
                    g1 = sbsm.tile([NE, 1024], F16, tag="g1")
                    nc.vector.scalar_tensor_tensor(
                        g1[:], xc16[:, cs], dp_sb[:, j:j + 1], yp[:, cs],
                        op0=OP.mult, op1=OP.add)
                    nc.vector.tensor_tensor(g16[:, cs], g1[:], sz16[:, cs],
                                            op=OP.mult)

                # ---- AllGather g ----
                ibg = dram.tile([NE, T], F16, tag="ib_g")
                obg = dram.tile([4 * NE, T], F16, tag="ob_g")
                nc.gpsimd.dma_start(ibg[:], g16[:])
                nc.gpsimd.collective_compute(
                    "AllGather", OP.bypass, replica_groups=GROUPS,
                    ins=[ibg.opt()], outs=[obg.opt()])
                gf = [sb1.tile([D, T], F16, tag=f"gf{et}", name=f"gf{et}")
                      for et in range(2)]
                for et in range(2):
                    nc.gpsimd.dma_start(gf[et][:],
                                        obg[128 * et:128 * (et + 1), :])

                # ---- outproj; residual update / final outputs ----
                for c in range(4):
                    cs = slice(512 * c, 512 * (c + 1))
                    op_ = ps.tile([D, 512], F32, tag="b1", name="op_")
                    for et in range(2):
                        nc.tensor.matmul(
                            op_[:],
                            wo_sb[:, (j * 2 + et) * D:(j * 2 + et + 1) * D],
                            gf[et][:, cs],
                            start=(et == 0), stop=(et == 1))
                    if not last:
                        nc.vector.tensor_tensor(r[:, cs], r[:, cs], op_[:],
                                                op=OP.add)
                    else:
                        ot = sb1.tile([D, 512], F32, tag="ot", bufs=2,
                                      name="ot")
                        nc.vector.tensor_tensor(ot[:], r[:, cs], op_[:],
                                                op=OP.add)
                        nm = "yf" if d == 0 else "yb"
                        nc.sync.dma_start(douts[nm][:, cs], ot[:])

        # ---------------- residual outputs ----------------
        nc.sync.dma_start(douts["rf"], res[0][:])
        nc.sync.dma_start(douts["rb"], res[1][:])

    nc.compile()
    return nc


def _host_inputs(inputs):
    """Build the 8 per-core input maps from the full problem inputs."""
    x = np.asarray(inputs["x"], np.float32)
    convd_w = np.asarray(inputs["convd_w"], np.float32)
    convd_b = np.asarray(inputs["convd_b"], np.float32)
    ln_w = np.asarray(inputs["ln_w"], np.float32)
    ln_b = np.asarray(inputs["ln_b"], np.float32)
    in_proj_w = np.asarray(inputs["in_proj_w"], np.float32)
    conv_w = np.asarray(inputs["conv_w"], np.float32)
    conv_b = np.asarray(inputs["conv_b"], np.float32)
    xproj_w = np.asarray(inputs["xproj_w"], np.float32)
    dtproj_w = np.asarray(inputs["dtproj_w"], np.float32)
    dtproj_b = np.asarray(inputs["dtproj_b"], np.float32)
    A_log = np.asarray(inputs["A_log"], np.float32)
    Dparam = np.asarray(inputs["Dparam"], np.float32)
    outproj_w = np.asarray(inputs["outproj_w"], np.float32)

    # [ci, k*D + co] = convd_w[co, ci, k]
    wconv_h = np.transpose(convd_w, (1, 2, 0)).reshape(CIN, 4 * D)
    wconv_h = np.ascontiguousarray(wconv_h).astype(np.float16)

    selb_h = np.zeros((40, 128), np.float16)
    selc_h = np.zeros((40, 128), np.float16)
    sely_h = np.zeros((D, NJ * NE), np.float16)
    for p in range(128):
        selb_h[8 + p % 16, p] = 1.0
        selc_h[24 + p % 16, p] = 1.0
        for jl in range(NJ):
            sely_h[p, jl * NE + 8 * jl + p // 16] = 1.0
    selr_h = np.zeros((NE, NJ * 128), np.float16)
    for jl in range(NJ):
        for m in range(128):
            selr_h[8 * jl + m // 16, jl * 128 + m] = 1.0

    in_maps = []
    for core in range(N_CORES):
        b, q = divmod(core, 4)
        e0 = NE * q
        sl = slice(e0, e0 + NE)

        wxk_h = np.zeros((D, 24 * NE), np.float16)
        wz_h = np.zeros((D, 6 * NE), np.float16)
        wxp_h = np.zeros((D, 12 * 40), np.float16)
        wdt_h = np.zeros((DTR, 6 * NE), np.float16)
        bdt_h = np.zeros((NE, 6), np.float32)
        asc_h = np.zeros((D, 6 * NJ), np.float32)
        dp_h = np.zeros((NE, 6), np.float32)
        lnw_h = np.zeros((D, 6), np.float32)
        lnb_h = np.zeros((D, 6), np.float32)
        wo_h = np.zeros((D, 12 * D), np.float16)
        cb_h = np.zeros((NE, 6), np.float32)

        for j in range(6):
            Wx = in_proj_w[j][:DI, :]          # (256, 128)
            for k in range(4):
                Mk = (Wx[sl, :] * conv_w[j][sl, k:k + 1]).T   # (128, 64)
                wxk_h[:, (j * 4 + k) * NE:(j * 4 + k + 1) * NE] = \
                    Mk.astype(np.float16)
            wz_h[:, j * NE:(j + 1) * NE] = \
                in_proj_w[j][DI + e0:DI + e0 + NE, :].T.astype(np.float16)
            xp = xproj_w[j].T                   # (256, 40)
            for et in range(2):
                wxp_h[:, (j * 2 + et) * 40:(j * 2 + et + 1) * 40] = \
                    xp[128 * et:128 * (et + 1), :].astype(np.float16)
            wdt_h[:, j * NE:(j + 1) * NE] = \
                dtproj_w[j][sl, :].T.astype(np.float16)
            bdt_h[:, j] = dtproj_b[j][sl]
            Aj = -np.exp(A_log[j])              # (256, 16)
            for jl in range(NJ):
                asc_h[:, j * NJ + jl] = Aj[e0 + 8 * jl:e0 + 8 * (jl + 1),
                                           :].reshape(128)
            dp_h[:, j] = Dparam[j][sl]
            lnw_h[:, j] = ln_w[j]
            lnb_h[:, j] = ln_b[j]
            po = outproj_w[j].T                 # (256, 128)
            for et in range(2):
                wo_h[:, (j * 2 + et) * D:(j * 2 + et + 1) * D] = \
                    po[128 * et:128 * (et + 1), :].astype(np.float16)
            cb_h[:, j] = conv_b[j][sl]

        in_maps.append({
            "x16": x[b].astype(np.float16),
            "wconv": wconv_h,
            "convb": convd_b[:, None].astype(np.float32),
            "wxk": wxk_h, "wz": wz_h, "wxp": wxp_h, "wdt": wdt_h,
            "bdt": bdt_h, "asc": asc_h, "dpp": dp_h,
            "lnw": lnw_h, "lnb": lnb_h, "wo": wo_h, "cbb": cb_h,
            "selr": selr_h, "selb": selb_h, "selc": selc_h, "sely": sely_h,
            "onesu": np.ones((1, D), np.float16),
            "onesc": np.ones((D, 1), np.float16),
            "oneb": np.ones((NE, 1), np.float32),
            "epsb": np.full((1, 1), 1e-5, np.float32),
        })
    return in_maps


def kernel(**inputs):
    if "nc" not in _CACHED:
        _CACHED["nc"] = _build_module()
    nc = _CACHED["nc"]
    in_maps = _host_inputs(inputs)
    res = run_bass_kernel_spmd(nc, in_maps, core_ids=list(range(N_CORES)))
    out = np.zeros((B, D, T), np.float32)
    rf = np.zeros((B, T, D), np.float32)
    rb = np.zeros((B, T, D), np.float32)
    for b in range(B):
        r = res.results[4 * b]
        out[b] = r["yf"] + r["yb"][:, ::-1]
        rf[b] = r["rf"].T
        rb[b] = r["rb"].T
    return out, rf, rb
